# revision 37
# baseline (speedup 1.0000x reference)
"""GraphConv (DGL norm='both' + ELU) Trainium2 kernel, 8-way SPMD.

  out = ELU( Din^{-1/2} * A * Dout^{-1/2} * h @ W + b )

Strategy (dst-node sharding, graph preprocessing on host):
  - Nodes are packed into 128-node "blocks" (serpentine over in-degree order
    for edge balance); 8 cores x (N_blocks/8) blocks; h replicated per core
    as bf16 (halves the random-gather traffic, the kernel's roofline term).
  - Per (block, src-window) group (int16 gather limit of 32767 rows -> 4
    windows of 25000 rows), in-edges are gathered with dma_gather (SWDGE
    descriptor gather, 512B/row, <=1024 idxs/call, trailing -1 pads skipped
    via a runtime count register) into SBUF, 128 edges per chunk.
  - Weighted segment-sum on the TensorEngine: for each 128-edge chunk a
    selection matrix S[p,d] = (dstcol[p]==d)*coef[p] is built in one DVE
    tensor_scalar op (iota compare + scale, bf16 out) and matmul'd into a
    PSUM accumulator agg[128 dst, 256 feat].
  - agg is scaled by Din^{-1/2} (ACT copy w/ per-partition scale), transposed
    via PE, multiplied by W (fp32) with the bias folded in as a K=1
    ones x bias matmul, and ELU'd (relu/exp on ACT + one fused DVE op).
  - Host un-permutes the 8 core outputs back to node order.
"""

import os
import sys
import time

import numpy as np

try:
    import concourse.bass as bass
except ImportError:  # fresh grading dir: concourse comes from the container env
    for _p in ("/opt/trn_rl_repo", "/root/.axon_site/_ro/trn_rl_repo"):
        if os.path.isdir(_p) and _p not in sys.path:
            sys.path.append(_p)
    import concourse.bass as bass

import ml_dtypes
import concourse.tile as tile
from concourse import bacc, mybir

# ---------------------------------------------------------------------------
# Problem config (hardcoded per the task statement)
# ---------------------------------------------------------------------------
N_NODES = 100000
DIM = 256
CORES = 8
WIN = 20000  # src gather window (int16 indices must stay < 32768)
P = 128
# HW limit: one dma_gather generates ceil(n/128)*128/16 + 1 descriptors per
# SDMA engine; the runtime's SWDGE descriptor ring caps a single call at
# 8 chunks (empirically 1024 idxs ok, 1152 crashes).
MAX_CALL_IDXS = 1024
SUB_CHUNKS = MAX_CALL_IDXS // P  # max chunks per dma_gather call

F32 = mybir.dt.float32
BF16 = mybir.dt.bfloat16
I16 = mybir.dt.int16
I32 = mybir.dt.int32

BF16_NP = ml_dtypes.bfloat16


class _Plan:
    """Host-side graph partitioning + per-core device input arrays."""

    def __init__(self, h, weight, bias, src, dst):
        n = h.shape[0]
        assert n == N_NODES and h.shape[1] == DIM
        e = src.shape[0]
        self.n_win = (n + WIN - 1) // WIN

        deg_out = np.bincount(src, minlength=n).astype(np.float32)
        deg_in = np.bincount(dst, minlength=n).astype(np.float32)
        a_src = 1.0 / np.sqrt(np.maximum(deg_out, 1.0))
        b_dst = 1.0 / np.sqrt(np.maximum(deg_in, 1.0))

        # --- node -> (block, pos): serpentine over degree order ---
        # G (blocks per gather call) depends on c_w which depends on the
        # assignment; iterate to a fixed point.
        bpc0 = (n + P * CORES - 1) // (P * CORES)
        bpc = bpc0
        self.G = 1
        for _ in range(4):
            nb = bpc * CORES
            order = np.argsort(-deg_in, kind="stable")
            padded = np.concatenate([order, np.full(nb * P - n, -1, np.int64)])
            grid = padded.reshape(P, nb)
            grid[1::2] = grid[1::2, ::-1]
            node_block = np.empty(n, np.int64)
            node_pos = np.empty(n, np.int64)
            mask = grid >= 0
            b_idx = np.broadcast_to(np.arange(nb), (P, nb))
            r_idx = np.broadcast_to(np.arange(P)[:, None], (P, nb))
            node_block[grid[mask]] = b_idx[mask]
            node_pos[grid[mask]] = r_idx[mask]

            eb = node_block[dst]
            ew = src // WIN
            key = eb * self.n_win + ew
            counts = np.bincount(key, minlength=nb * self.n_win)
            self.c_w = max(1, int(-(-counts.max() // P)))
            # Multi-block calls need dup-pads (mid-call -1 is illegal); they
            # amortize the ~1us SWDGE issue cost across more descriptors.
            g_new = max(1, SUB_CHUNKS // self.c_w)
            bpc_new = -(-bpc0 // g_new) * g_new
            if bpc_new == bpc and g_new == self.G:
                break
            self.G, bpc = g_new, bpc_new
        self.bpc = bpc
        self.grid = grid  # [P, nb]; grid[r, b] = node id or -1

        perm = np.argsort(key, kind="stable")
        s_src = src[perm]
        s_dst = dst[perm]
        self.c_b = self.c_w * self.n_win  # chunks per block
        slots = self.c_w * P  # slots per (block, win) group

        # --- per-(block,win) slot arrays; pads duplicate the last real idx ---
        n_grp = nb * self.n_win
        idx_flat = np.zeros((n_grp, slots), np.int16)
        dstcol = np.zeros((n_grp, slots), np.float32)
        coef = np.zeros((n_grp, slots), np.float32)
        starts = np.zeros(n_grp + 1, np.int64)
        np.cumsum(counts, out=starts[1:])
        within = np.arange(e) - starts[key[perm]]
        gk = key[perm]
        idx_flat[gk, within] = (s_src % WIN).astype(np.int16)
        dstcol[gk, within] = node_pos[s_dst].astype(np.float32)
        coef[gk, within] = a_src[s_src]
        pad_mask = np.broadcast_to(np.arange(slots), (n_grp, slots)) >= counts[:, None]
        lastv = np.zeros(n_grp, np.int16)
        nz = counts > 0
        lastv[nz] = idx_flat[nz, counts[nz] - 1]
        dup_padded = np.where(pad_mask, lastv[:, None], idx_flat)
        neg_padded = np.where(pad_mask, np.int16(-1), idx_flat)
        if DUPPAD:
            idx_flat = dup_padded
        elif self.G > 1:
            # hybrid: a call concats G blocks' groups; only the LAST group's
            # pads are trailing, so dup-pad the first G-1 and -1-pad the last
            # (skipped via the runtime count register).
            g_pos = (np.arange(n_grp) // self.n_win) % self.G
            idx_flat = np.where((g_pos == self.G - 1)[:, None], neg_padded, dup_padded)
        else:
            idx_flat = neg_padded
        self.call_counts = counts.astype(np.int32)

        # --- per-core layouts ---
        # idx for call (sb, w) = concat of G blocks' (b, w) groups.
        # SBUF idx tile [128, n_calls * G*c_w*8]: (p, call_off + s) = L[s*16 + p%16]
        # Each (block, window) group is gathered in ceil(c_w/SUB_CHUNKS)
        # sub-calls of <= 1024 idxs.
        self.n_sub = -(-self.c_w // SUB_CHUNKS)
        assert self.G == 1 or self.n_sub == 1
        n_grps = bpc * self.n_win  # groups (not sub-calls) per core
        g = self.G
        spc = bpc // g
        L = idx_flat.reshape(CORES, spc, g, self.n_win, slots)
        L = np.ascontiguousarray(L.transpose(0, 1, 3, 2, 4))
        L = L.reshape(CORES, n_grps, self.c_w * 8, 16)
        idx_sb = np.ascontiguousarray(
            np.broadcast_to(
                L.transpose(0, 3, 1, 2)[:, None, :, :, :],
                (CORES, 8, 16, n_grps, self.c_w * 8),
            ).reshape(CORES, P, n_grps * self.c_w * 8)
        )
        self.idx_sb = idx_sb
        if g > 1:
            # one call per (superblock, window): first G-1 groups fully
            # gathered (dup-padded), last group counted exactly
            last_cnt = self.call_counts.reshape(CORES, spc, g, self.n_win)[:, :, -1, :]
            call_cnt = (g - 1) * slots + last_cnt
            self.counts_sb = np.ascontiguousarray(
                call_cnt.reshape(CORES, 1, spc * self.n_win)
            ).astype(np.int32)
        else:
            # per-sub-call valid counts: clip(group_count - 1024*j, 0, sub_slots_j)
            cc = self.call_counts.reshape(CORES, n_grps, 1)
            j = np.arange(self.n_sub)
            sub_slots = np.minimum(self.c_w - j * SUB_CHUNKS, SUB_CHUNKS) * P
            sub_counts = np.clip(cc - j * MAX_CALL_IDXS, 0, sub_slots)
            self.counts_sb = np.ascontiguousarray(
                sub_counts.reshape(CORES, 1, n_grps * self.n_sub)
            ).astype(np.int32)

        # per-slot arrays [128, bpc*c_b]: (p, b*c_b + w*c_w + c) = val[slot c*128+p]
        def slot_layout(v):
            v = v.reshape(CORES, bpc * self.c_b, P)
            return np.ascontiguousarray(v.transpose(0, 2, 1))

        self.dstcol_sb = slot_layout(dstcol)
        self.coef_sb = slot_layout(coef)

        bd = np.ones((P, nb), np.float32)
        bd[mask] = b_dst[grid[mask]]
        self.bdst_sb = np.ascontiguousarray(bd.reshape(P, CORES, bpc).transpose(1, 0, 2))
        self.iota = np.ascontiguousarray(
            np.broadcast_to(np.arange(P, dtype=np.float32), (P, P))
        )
        self.ident = np.eye(P, dtype=np.float32)
        self.weight = np.ascontiguousarray(weight, np.float32)
        self.bias = np.ascontiguousarray(bias, np.float32).reshape(1, DIM)
        self.h_bf16 = np.ascontiguousarray(h, np.float32).astype(BF16_NP)

    def in_maps(self):
        maps = []
        for k in range(CORES):
            maps.append(
                {
                    "h": self.h_bf16,
                    "weight": self.weight,
                    "bias": self.bias,
                    "iota": self.iota,
                    "ident": self.ident,
                    "idx": self.idx_sb[k],
                    "dstcol": self.dstcol_sb[k],
                    "coef": self.coef_sb[k],
                    "bdst": self.bdst_sb[k],
                    "counts": self.counts_sb[k],
                }
            )
        return maps

    def assemble(self, results):
        out = np.empty((N_NODES, DIM), np.float32)
        for k in range(CORES):
            rows = results[k]["out"].reshape(self.bpc, P, DIM)
            g = self.grid[:, k * self.bpc : (k + 1) * self.bpc]  # [P, bpc]
            m = g >= 0
            out[g.T[m.T]] = rows[m.T]
        return out


STAGE = int(os.environ.get("K_STAGE", "3"))  # 1=gather only, 2=+segsum, 3=full
DUPPAD = bool(int(os.environ.get("K_DUPPAD", "0")))


def _build_program(plan):
    """Trace the SPMD Tile program (identical across cores)."""
    nc = bacc.Bacc(
        "TRN2",
        target_bir_lowering=False,
        debug=False,
        num_devices=CORES,
        num_swdge_queues=4,
    )
    bpc, c_w, c_b, n_win = plan.bpc, plan.c_w, plan.c_b, plan.n_win
    G = plan.G
    n_sub = plan.n_sub
    grp_i16 = c_w * 8  # idx free-dim columns per (block, window) group
    n_calls = (bpc // G) * n_win * n_sub

    h = nc.dram_tensor("h", [N_NODES, DIM], BF16, kind="ExternalInput").ap()
    weight = nc.dram_tensor("weight", [DIM, DIM], F32, kind="ExternalInput").ap()
    biasrow = nc.dram_tensor("bias", [1, DIM], F32, kind="ExternalInput").ap()
    iota_d = nc.dram_tensor("iota", [P, P], F32, kind="ExternalInput").ap()
    ident_d = nc.dram_tensor("ident", [P, P], F32, kind="ExternalInput").ap()
    idx_d = nc.dram_tensor(
        "idx", [P, bpc * n_win * grp_i16], I16, kind="ExternalInput"
    ).ap()
    dstcol_d = nc.dram_tensor("dstcol", [P, bpc * c_b], F32, kind="ExternalInput").ap()
    coef_d = nc.dram_tensor("coef", [P, bpc * c_b], F32, kind="ExternalInput").ap()
    bdst_d = nc.dram_tensor("bdst", [P, bpc], F32, kind="ExternalInput").ap()
    counts_d = nc.dram_tensor("counts", [1, n_calls], I32, kind="ExternalInput").ap()
    out_d = nc.dram_tensor("out", [bpc * P, DIM], F32, kind="ExternalOutput").ap()

    with tile.TileContext(nc) as tc:
        with (
            tc.tile_pool(name="resident", bufs=1) as res,
            tc.tile_pool(name="edges", bufs=4) as epool,
            tc.tile_pool(name="work", bufs=3) as wpool,
            tc.tile_pool(name="spool", bufs=4) as spool,
            tc.tile_pool(name="psum", bufs=2, space="PSUM") as ppool,
        ):
            # resident tiles
            iota_t = res.tile([P, P], F32)
            nc.sync.dma_start(iota_t[:], iota_d[:])
            ident = res.tile([P, P], F32)
            nc.sync.dma_start(ident[:], ident_d[:])
            w_t = res.tile([P, 2, DIM], F32)
            nc.sync.dma_start(w_t[:, 0, :], weight[0:P, :])
            nc.sync.dma_start(w_t[:, 1, :], weight[P:DIM, :])
            bias_t = res.tile([1, DIM], F32)
            nc.sync.dma_start(bias_t[:], biasrow[:])
            ones_t = res.tile([1, P], F32)
            nc.vector.memset(ones_t[:], 1.0)
            idx_t = res.tile([P, bpc * n_win * grp_i16], I16)
            nc.sync.dma_start(idx_t[:], idx_d[:])
            dstcol_t = res.tile([P, bpc * c_b], F32)
            nc.sync.dma_start(dstcol_t[:], dstcol_d[:])
            coef_t = res.tile([P, bpc * c_b], F32)
            nc.sync.dma_start(coef_t[:], coef_d[:])
            bdst_t = res.tile([P, bpc], F32)
            nc.sync.dma_start(bdst_t[:], bdst_d[:])
            counts_t = res.tile([1, n_calls], I32)
            nc.sync.dma_start(counts_t[:], counts_d[:])

            cnt_regs = [nc.gpsimd.alloc_register(f"cnt{i}") for i in range(4)]

            for sb in range(bpc // G):
                # edge buffer: chunk (g, w, c) lives at column w*(G*c_w) + g*c_w + c
                ebuf = epool.tile([P, n_win * G * c_w, DIM], BF16, tag="ebuf")
                if sb < 4:
                    nc.vector.memset(ebuf[:], 0.0)
                for w in range(n_win):
                    lo = w * WIN
                    hi = min(lo + WIN, N_NODES)
                    for j in range(n_sub):
                        k = (sb * n_win + w) * n_sub + j
                        sub_c = min(c_w - j * SUB_CHUNKS, SUB_CHUNKS) * G
                        if DUPPAD:
                            reg = sub_c * P
                        else:
                            reg = cnt_regs[k % 4]
                            nc.gpsimd.load(reg, counts_t[0:1, k : k + 1])
                        c0 = j * SUB_CHUNKS
                        i16_0 = (sb * n_win + w) * G * grp_i16 + c0 * 8 * G
                        nc.gpsimd.dma_gather(
                            ebuf[:, w * (G * c_w) + c0 * G : w * (G * c_w) + c0 * G + sub_c, :],
                            h[lo:hi, :],
                            idx_t[:, i16_0 : i16_0 + sub_c * 8],
                            sub_c * P,
                            reg,
                            DIM,
                            queue_num=k % 4,
                        )

                for g_i in range(G):
                    b = sb * G + g_i
                    if STAGE == 1:
                        o_t = wpool.tile([P, DIM], F32, tag="out")
                        nc.vector.tensor_copy(o_t[:], ebuf[:, g_i * c_w, :])
                        nc.sync.dma_start(out_d[b * P : (b + 1) * P, :], o_t[:])
                        continue
                    # --- weighted segment-sum via PE ---
                    agg_ps = ppool.tile([P, DIM], F32, tag="agg")
                    for ci in range(c_b):
                        s_t = spool.tile([P, P], BF16, tag="sel")
                        nc.vector.tensor_scalar(
                            s_t[:],
                            iota_t[:],
                            dstcol_t[:, b * c_b + ci : b * c_b + ci + 1],
                            coef_t[:, b * c_b + ci : b * c_b + ci + 1],
                            mybir.AluOpType.is_equal,
                            mybir.AluOpType.mult,
                        )
                        w_i, c_i = divmod(ci, c_w)
                        nc.tensor.matmul(
                            agg_ps[:],
                            lhsT=s_t[:],
                            rhs=ebuf[:, w_i * (G * c_w) + g_i * c_w + c_i, :],
                            start=(ci == 0),
                            stop=(ci == c_b - 1),
                        )

                    # --- scale by Din^{-1/2}, transpose, @W + bias, ELU ---
                    agg_sb = wpool.tile([P, DIM], F32, tag="aggsb")
                    nc.scalar.activation(
                        agg_sb[:],
                        agg_ps[:],
                        mybir.ActivationFunctionType.Copy,
                        scale=bdst_t[:, b : b + 1],
                    )
                    if STAGE == 2:
                        nc.sync.dma_start(out_d[b * P : (b + 1) * P, :], agg_sb[:])
                        continue
                    aggT_ps = ppool.tile([P, DIM], F32, tag="aggT")
                    nc.tensor.transpose(aggT_ps[:, 0:P], agg_sb[:, 0:P], ident[:])
                    nc.tensor.transpose(aggT_ps[:, P:DIM], agg_sb[:, P:DIM], ident[:])
                    aggT_sb = wpool.tile([P, DIM], F32, tag="aggTsb")
                    nc.scalar.activation(
                        aggT_sb[:], aggT_ps[:], mybir.ActivationFunctionType.Copy
                    )

                    z_ps = ppool.tile([P, DIM], F32, tag="z")
                    nc.tensor.matmul(
                        z_ps[:], lhsT=ones_t[:], rhs=bias_t[:], start=True, stop=False
                    )
                    nc.tensor.matmul(
                        z_ps[:],
                        lhsT=aggT_sb[:, 0:P],
                        rhs=w_t[:, 0, :],
                        start=False,
                        stop=False,
                    )
                    nc.tensor.matmul(
                        z_ps[:],
                        lhsT=aggT_sb[:, P:DIM],
                        rhs=w_t[:, 1, :],
                        start=False,
                        stop=True,
                    )

                    # ELU(z) = relu(z) + exp(-relu(-z)) - 1
                    r_t = wpool.tile([P, DIM], F32, tag="relu")
                    nc.scalar.activation(
                        r_t[:], z_ps[:], mybir.ActivationFunctionType.Relu
                    )
                    rn_t = wpool.tile([P, DIM], F32, tag="rneg")
                    nc.scalar.activation(
                        rn_t[:], z_ps[:], mybir.ActivationFunctionType.Relu, scale=-1.0
                    )
                    e_t = wpool.tile([P, DIM], F32, tag="exp")
                    nc.scalar.activation(
                        e_t[:], rn_t[:], mybir.ActivationFunctionType.Exp, scale=-1.0
                    )
                    o_t = wpool.tile([P, DIM], F32, tag="out")
                    nc.vector.scalar_tensor_tensor(
                        o_t[:],
                        r_t[:],
                        -1.0,
                        e_t[:],
                        mybir.AluOpType.add,
                        mybir.AluOpType.add,
                    )
                    nc.sync.dma_start(out_d[b * P : (b + 1) * P, :], o_t[:])

    nc.compile()
    return nc


# ---------------------------------------------------------------------------
# Execution via PJRT on the axon-tunneled NeuronCores (adapted from
# concourse.bass2jax.run_bass_via_pjrt, pinned to the axon/neuron platform).
# ---------------------------------------------------------------------------
_EXEC_CACHE = {}


def _axon_devices():
    import jax

    try:
        return jax.devices("axon")
    except RuntimeError:
        return jax.devices()


def _make_executor(nc):
    import jax
    import numpy as _np
    from jax.sharding import Mesh, PartitionSpec
    from jax.experimental.shard_map import shard_map
    from concourse import bass2jax
    from concourse import mybir as mb

    bass2jax.install_neuronx_cc_hook()
    partition_name = nc.partition_id_tensor.name if nc.partition_id_tensor else None

    in_names, out_names, out_avals, zero_outs = [], [], [], []
    for alloc in nc.m.functions[0].allocations:
        if not isinstance(alloc, mb.MemoryLocationSet):
            continue
        name = alloc.memorylocations[0].name
        if alloc.kind == "ExternalInput":
            if name != partition_name:
                in_names.append(name)
        elif alloc.kind == "ExternalOutput":
            out_names.append(name)
            shape = tuple(alloc.tensor_shape)
            dtype = mb.dt.np(alloc.dtype)
            out_avals.append(jax.core.ShapedArray(shape, dtype))
            zero_outs.append(_np.zeros(shape, dtype))
    n_params = len(in_names)
    n_outs = len(out_avals)
    all_names = in_names + out_names + ([partition_name] if partition_name else [])

    def _body(*args):
        operands = list(args)
        if partition_name is not None:
            operands.append(bass2jax.partition_id_tensor())
        outs = bass2jax._bass_exec_p.bind(
            *operands,
            out_avals=tuple(out_avals),
            in_names=tuple(all_names),
            out_names=tuple(out_names),
            lowering_input_output_aliases=(),
            sim_require_finite=True,
            sim_require_nnan=True,
            nc=nc,
        )
        return tuple(outs)

    devices = _axon_devices()[:CORES]
    assert len(devices) == CORES, f"need {CORES} axon devices, got {len(devices)}"
    mesh = Mesh(np.asarray(devices), ("core",))
    in_specs = (PartitionSpec("core"),) * (n_params + n_outs)
    out_specs = (PartitionSpec("core"),) * n_outs
    fn = jax.jit(
        shard_map(
            _body, mesh=mesh, in_specs=in_specs, out_specs=out_specs, check_rep=False
        ),
        keep_unused=True,
    )
    return fn, in_names, out_names, zero_outs, mesh


def _execute(nc, in_maps, time_iters=0):
    key = id(nc)
    if key not in _EXEC_CACHE:
        _EXEC_CACHE.clear()
        _EXEC_CACHE[key] = _make_executor(nc)
    fn, in_names, out_names, zero_outs, mesh = _EXEC_CACHE[key]

    concat_in = [
        np.concatenate([np.asarray(in_maps[c][n]) for c in range(CORES)], axis=0)
        for n in in_names
    ]
    concat_zero = [np.concatenate([z for _ in range(CORES)], axis=0) for z in zero_outs]
    args = concat_in + concat_zero
    outs = fn(*args)
    outs = [np.asarray(o) for o in outs]

    exec_ns = None
    if time_iters:
        import jax
        from jax.sharding import NamedSharding, PartitionSpec

        shard = NamedSharding(mesh, PartitionSpec("core"))
        dargs = [jax.device_put(a, shard) for a in args]
        jax.block_until_ready(fn(*dargs))
        times = []
        for _ in range(time_iters):
            t0 = time.perf_counter()
            r = fn(*dargs)
            jax.block_until_ready(r)
            times.append(time.perf_counter() - t0)
        exec_ns = min(times) * 1e9

    results = []
    for c in range(CORES):
        m = {}
        for i, nme in enumerate(out_names):
            per = outs[i].shape[0] // CORES
            m[nme] = outs[i][c * per : (c + 1) * per]
        results.append(m)
    return results, exec_ns


_PROGRAM_CACHE = {}


def _get_plan_and_program(h, weight, bias, src, dst):
    plan = _Plan(h, weight, bias, src, dst)
    pkey = (plan.bpc, plan.c_w, plan.n_win)
    if pkey not in _PROGRAM_CACHE:
        _PROGRAM_CACHE.clear()
        _PROGRAM_CACHE[pkey] = _build_program(plan)
    return plan, _PROGRAM_CACHE[pkey]


def kernel(h, weight, bias, src, dst, _time_iters=0):
    h = np.asarray(h, np.float32)
    weight = np.asarray(weight, np.float32)
    bias = np.asarray(bias, np.float32)
    src = np.asarray(src, np.int32)
    dst = np.asarray(dst, np.int32)
    plan, nc = _get_plan_and_program(h, weight, bias, src, dst)
    results, exec_ns = _execute(nc, plan.in_maps(), time_iters=_time_iters)
    out = plan.assemble(results)
    if _time_iters:
        kernel.last_exec_ns = exec_ns
    return out
